# revision 1
# baseline (speedup 1.0000x reference)
"""Trainium2 Bass kernel: 3x3 same-padding conv2d, NCHW.

Full inputs: x (32, 64, 112, 112) f32, W (64, 128, 3, 3) f32 (IOHW).
Full output: (32, 128, 112, 112) f32.

Strategy: data-parallel over batch across 8 NeuronCores (4 images/core).
The PE cost of a matmul is proportional to the output free size N only
(independent of K), so the kernel packs the cin=64 x 9-tap contraction
into 5 matmuls per 4-row output tile (the K<=128 floor: ceil(576/128)):

  tileA [128, 113, 114] f16: partitions 0-63 = padded image rows 0..112,
        partitions 64-127 = the same image shifted down one row (1..113).
  tileB [128, 112, 114] f16: partitions 0-63 = rows 2..113, partitions
        64-127 = rows 2..113 shifted one column. Built on-chip by DVE
        from tileA (partition-aligned copies), so it costs no HBM traffic.

  t0-t2: K=128 pairs (u=0,v)+(u=1,v) from tileA
  t3:    K=64  single (u=2,v=2) from tileA's shifted half
  t4:    K=128 pair (u=2,v=0)+(u=2,v=1) from tileB

Inputs stream as f16 and outputs DMA out as f16 (PSUM accumulates f32;
total quantization error ~6e-4 of absmax vs the 2e-2 gate), halving both
directions of HBM traffic. PSUM->SBUF conversion copies run on the
Activation engine; output slab DMAs issue from Pool's SWDGE (cheap SEQ)
and input DMAs from SP, so no engine sequencer becomes a serial
bottleneck and the two DMA streams interleave on the DMA engines.
"""

import numpy as np

import concourse.bacc as bacc
import concourse.tile as tile
import concourse.mybir as mybir
from concourse.bass_utils import run_bass_kernel_spmd

F32 = mybir.dt.float32
F16 = mybir.dt.float16

NCORES = 8
H = W_ = 112
HP = H + 1  # 113 rows per shifted copy
WP = W_ + 2  # 114 padded cols
NCHUNK = 5
NTILE = H // 4  # 28 output-row tiles of 4 rows x 112 cols = 448

# input row chunks: small head so the PE starts early
BOUNDS = [0, 4, 8, 14, 24, 40, 58, 78, 96, HP]

_NC_CACHE = []


def _build(repeat=1):
    nc = bacc.Bacc()
    xp_ext = nc.declare_dram_parameter("xp", [4, 128, HP, WP], F16, isOutput=False)
    wt_ext = nc.declare_dram_parameter("wt", [128, NCHUNK * 128], F16, isOutput=False)
    out_ext = nc.declare_dram_parameter("out", [4, 128, H, W_], F16, isOutput=True)

    with tile.TileContext(nc) as tc:
        with (
            tc.tile_pool(name="xpool", bufs=2) as xpool,
            tc.tile_pool(name="bpool", bufs=2) as bpool,
            tc.tile_pool(name="wpool", bufs=1) as wpool,
            tc.tile_pool(name="opool", bufs=8) as opool,
            tc.tile_pool(name="psum", bufs=8, space="PSUM") as psum,
        ):
            wt = wpool.tile([128, NCHUNK * 128], F16)
            nc.sync.dma_start(out=wt[:], in_=wt_ext[:])

            for img in [i for _ in range(repeat) for i in range(4)]:
                xt = xpool.tile([128, HP, WP], F16)
                xb = bpool.tile([128, H, WP], F16)
                for ci, (r_s, r_e) in enumerate(zip(BOUNDS[:-1], BOUNDS[1:])):
                    # the run's head chunk goes via Pool's SWDGE so its
                    # descriptor gen overlaps the weight DMA's HWDGE gen
                    in_eng = nc.gpsimd if (img == 0 and ci == 0) else nc.sync
                    in_eng.dma_start(
                        out=xt[:, r_s:r_e, :], in_=xp_ext[img, :, r_s:r_e, :]
                    )
                    # xb[p<64, r, c] = xt[p<64, r+2, c]   (= xpad rows 2..)
                    # xb[p>=64, r, c] = xt[p>=64, r+1, c+1] (rows 2.., cols 1..)
                    lo_s, lo_e = max(0, r_s - 2), max(0, r_e - 2)
                    if lo_e > lo_s:
                        nc.vector.tensor_copy(
                            xb[0:64, lo_s:lo_e, :], xt[0:64, lo_s + 2 : lo_e + 2, :]
                        )
                    hi_s, hi_e = max(0, r_s - 1), min(H, r_e - 1)
                    if hi_e > hi_s:
                        nc.vector.tensor_copy(
                            xb[64:128, hi_s:hi_e, 0:113],
                            xt[64:128, hi_s + 1 : hi_e + 1, 1:114],
                        )
                # bottom pad row of the lo half (xpad row 113 = zeros)
                nc.vector.memset(xb[0:64, 111:112, :], 0.0)

                for ti in range(NTILE):
                    r0 = 4 * ti
                    ps = psum.tile([128, 4, 112], F32)
                    lr = 3 if ti == NTILE - 1 else 4  # u=2 rows (row 111: pad)
                    # v=1 first (start, untrimmed, initializes all of ps);
                    # v=0 skips output col 0, v=2 skips col 111 (pad-zero taps)
                    nc.tensor.matmul(
                        ps[:],
                        wt[:, 128:256],
                        xt[:, r0 : r0 + 4, 1 : 1 + W_],
                        start=True,
                        stop=False,
                    )
                    nc.tensor.matmul(
                        ps[:, :, 1:112],
                        wt[:, 0:128],
                        xt[:, r0 : r0 + 4, 1:112],
                        start=False,
                        stop=False,
                    )
                    nc.tensor.matmul(
                        ps[:, :, 0:111],
                        wt[:, 256:384],
                        xt[:, r0 : r0 + 4, 2:113],
                        start=False,
                        stop=False,
                    )
                    if img == 0 and ti == 0:
                        ps0 = ps
                        ot = opool.tile([128, 896], F16)
                        continue
                    if img == 0 and ti == 1:
                        for dps, dr0 in ((ps0, 0),):
                            nc.tensor.matmul(
                                dps[:, :, 0:111],
                                wt[64:128, 3 * 128 : 4 * 128],
                                xt[64:128, dr0 + 1 : dr0 + 5, 2:113],
                                start=False,
                                stop=False,
                            )
                            nc.tensor.matmul(
                                dps[:],
                                wt[:, 4 * 128 : 5 * 128],
                                xb[:, dr0 : dr0 + 4, 0:W_],
                                start=False,
                                stop=True,
                            )
                    # (u=2, v=2) single on tileA's shifted half
                    nc.tensor.matmul(
                        ps[:, 0:lr, 0:111],
                        wt[64:128, 3 * 128 : 4 * 128],
                        xt[64:128, r0 + 1 : r0 + 1 + lr, 2:113],
                        start=False,
                        stop=False,
                    )
                    # (u=2, v=0)+(u=2, v=1) pair on tileB
                    nc.tensor.matmul(
                        ps[:, 0:lr, :],
                        wt[:, 4 * 128 : 5 * 128],
                        xb[:, r0 : r0 + lr, 0:W_],
                        start=False,
                        stop=True,
                    )
                    # two psum tiles accumulate into one 8-row slab; Pool's
                    # SWDGE issues the slab DMA so Act's SEQ only runs copies
                    if ti % 2 == 0 or (img == 0 and ti == 1):
                        if not (img == 0 and ti == 1):
                            ot = opool.tile([128, 896], F16)
                    if img == 0 and ti == 1:
                        nc.scalar.copy(ot[:, 0:448], ps0[:])
                    copy_eng = (
                        nc.vector.tensor_copy
                        if (img == 3 and ti >= NTILE - 2)
                        else nc.scalar.copy
                    )
                    copy_eng(ot[:, (ti % 2) * 448 : (ti % 2 + 1) * 448], ps[:])
                    if img == 3 and ti >= NTILE - 2:
                        # drain the run's last slab as two 4-row HWDGE DMAs
                        # (shorter latency chain than Pool's SWDGE)
                        nc.sync.dma_start(
                            out=out_ext[img, :, r0 : r0 + 4, :],
                            in_=ot[:, (ti % 2) * 448 : (ti % 2 + 1) * 448],
                        )
                    elif ti % 2 == 1:
                        nc.gpsimd.dma_start(
                            out=out_ext[img, :, r0 - 4 : r0 + 4, :], in_=ot[:]
                        )
    nc.finalize()
    return nc


def get_nc():
    if not _NC_CACHE:
        _NC_CACHE.append(_build())
    return _NC_CACHE[0]


def make_in_maps(x, W):
    x = np.ascontiguousarray(np.asarray(x, dtype=np.float32))
    W = np.ascontiguousarray(np.asarray(W, dtype=np.float32))
    # lhsT per chunk: [K, cout]. W is (cin, cout, u, v).
    wt = np.zeros((128, NCHUNK * 128), dtype=np.float16)
    for t in range(3):
        wt[0:64, t * 128 : (t + 1) * 128] = W[:, :, 0, t]
        wt[64:128, t * 128 : (t + 1) * 128] = W[:, :, 1, t]
    wt[64:128, 3 * 128 : 4 * 128] = W[:, :, 2, 2]
    wt[0:64, 4 * 128 : 5 * 128] = W[:, :, 2, 0]
    wt[64:128, 4 * 128 : 5 * 128] = W[:, :, 2, 1]
    in_maps = []
    for c in range(NCORES):
        xs = x[c * 4 : (c + 1) * 4]  # [4, 64, 112, 112]
        xpad = np.zeros((4, 64, H + 2, WP), dtype=np.float16)
        xpad[:, :, 1 : H + 1, 1 : W_ + 1] = xs
        xp = np.empty((4, 128, HP, WP), dtype=np.float16)
        xp[:, 0:64] = xpad[:, :, 0:HP]          # rows 0..112 of padded image
        xp[:, 64:128] = xpad[:, :, 1 : HP + 1]  # rows 1..113 (shift by one)
        in_maps.append({"xp": xp, "wt": wt})
    return in_maps


def kernel(x, W):
    nc = get_nc()
    in_maps = make_in_maps(x, W)
    res = run_bass_kernel_spmd(nc, in_maps, list(range(NCORES)))
    out = np.concatenate(
        [res.results[c]["out"].astype(np.float32) for c in range(NCORES)], axis=0
    )
    return out

